# revision 2
# baseline (speedup 1.0000x reference)
"""Binary 3-layer CNN (sign activations + sign weights) on 8 NeuronCores.

Strategy: pure data parallel — 64 images -> 8 cores x 8 images.
Per core: 2 batches of 4 images; SBUF partition layout [128 = (4 img, 32 ch)].

Substrate cost model (measured):
 - ~70-120us once per run per static non-DMA instruction (size-independent)
   -> repetition lives in For_i hardware loops with register-offset APs;
 - ~150us per loop iteration (all-engine barrier) and ~10-15us per dynamic
   instruction execution across the 8 cores (engine work is time-sliced)
   -> few fat loops, minimum instruction executions;
 - a DMA whose completion gates later work stalls it ~1ms; scattered DMA
   rows cost ~2-3us each -> all DMAs contiguous, few, and off the
   critical path.
Design: h0/h1 feature planes are RESIDENT IN SBUF ([128, 258, 258] fp8);
conv0 reads a full sign(x) plane [4, 258, 258] loaded by ONE contiguous
DMA per batch; conv2 drains through a [4, 64, 256] f16 tile, one
contiguous DMA per 64 rows. The conv0-input / stage-0 / conv2-output
tiles share one SBUF buffer (disjoint lifetimes, same pool tag).
Math (identical to the exact unrolled baseline): all convs use fp8
operands (+-1/0, exact); tap-PAIRS packed into DoubleRow matmuls -> 5
matmuls per 512-f32 PSUM bank (2 out rows x 256 cols); the rhs k-tile dim
is a hand-built access pattern whose stride is the inter-tap offset.
conv0 contracts over K=4 image partitions (block-diagonal weights).
fp32 PSUM accumulation of +-1 values is exact.
"""

import numpy as np
import ml_dtypes

import concourse.mybir as mybir
import concourse.tile as tile
from concourse import bacc
from concourse.bass import ds
from concourse.bass_types import AP as RawAP
from concourse.bass_utils import run_bass_kernel_spmd

FP8 = mybir.dt.float8e4
F32 = mybir.dt.float32
F16 = mybir.dt.float16
AF = mybir.ActivationFunctionType
DR = mybir.MatmulPerfMode.DoubleRow

N_CORES = 8
IMG_PER_CORE = 8
B = 4          # images per partition-batch
H = W = 256
WP = 258       # padded width
HP = 258       # padded height
NB = IMG_PER_CORE // B
NBANK = 4      # psum banks filled per loop iteration (2 out rows each)
RPI = 2 * NBANK  # out rows per inner iteration
RC = 64        # conv2 output rows buffered per DMA
TAPS = [(t // 3, t % 3) for t in range(9)]


def _conv_taps_dr(nc, ps_slice, s_t, hin, r, npart=128):
    """9-tap conv into one 512-f32 psum bank slice: 4 DoubleRow + 1 single.
    hin is a padded fp8 plane [npart, rows, WP]; r is the out row (may be a
    dynamic ScalarValue); s_t is [npart, 9, 128]."""
    for pi, t0 in enumerate((0, 2, 4, 6)):
        dy0, dx0 = TAPS[t0]
        dy1, dx1 = TAPS[t0 + 1]
        delta = (dy1 - dy0) * WP + (dx1 - dx0)
        base = hin[:, ds(r + dy0, 2), dx0:dx0 + 256]
        rhs = RawAP(tensor=base.tensor,
                    ap=[[base.ap[0][0], npart], [delta, 2], [WP, 2], [1, 256]],
                    offset=base.offset)
        nc.tensor.matmul(ps_slice, s_t[:, t0:t0 + 2, :], rhs,
                         start=(pi == 0), stop=False, perf_mode=DR)
    nc.tensor.matmul(ps_slice, s_t[:, 8, :], hin[:, ds(r + 2, 2), 2:WP],
                     start=False, stop=True)


def _build_program(stages=('0', 'A', 'B', 'C')):
    nc = bacc.Bacc("TRN2", target_bir_lowering=False, debug=False)

    x_in = nc.dram_tensor("x", [IMG_PER_CORE, H, W], F32, kind="ExternalInput")
    s0_in = nc.dram_tensor("s0", [B, 9, 128], FP8, kind="ExternalInput")
    s1_in = nc.dram_tensor("s1", [128, 9, 128], FP8, kind="ExternalInput")
    s2_in = nc.dram_tensor("s2", [128, 9, 128], FP8, kind="ExternalInput")
    out_d = nc.dram_tensor("out", [NB, B, H, W], F16, kind="ExternalOutput")

    xs_d = nc.dram_tensor("xs", [IMG_PER_CORE, HP, WP], FP8)

    with tile.TileContext(nc) as tc:
        with (
            tc.tile_pool(name="const", bufs=1) as cpool,
            tc.tile_pool(name="shared", bufs=1) as shpool,
            tc.tile_pool(name="xprep", bufs=1) as xpool,
            tc.tile_pool(name="h0p", bufs=1) as h0pool,
            tc.tile_pool(name="h1p", bufs=1) as h1pool,
            tc.tile_pool(name="psum", bufs=1, space="PSUM") as pspool,
        ):
            # --- constants ---
            s0t = cpool.tile([B, 9, 128], FP8, tag="s0")
            nc.sync.dma_start(out=s0t[:, :, :], in_=s0_in[:, :, :])
            s1t = cpool.tile([128, 9, 128], FP8, tag="s1")
            nc.sync.dma_start(out=s1t[:, :, :], in_=s1_in[:, :, :])
            s2t = cpool.tile([128, 9, 128], FP8, tag="s2")
            nc.sync.dma_start(out=s2t[:, :, :], in_=s2_in[:, :, :])
            ztw = cpool.tile([1, WP], FP8, tag="ztw")
            nc.gpsimd.memset(ztw[:, :], 0.0)

            # --- pre-zero xs_d pad rows (independent static DMAs) ---
            for img in range(IMG_PER_CORE):
                nc.scalar.dma_start(out=xs_d[img, 0:1, :], in_=ztw[:, :])
                nc.scalar.dma_start(out=xs_d[img, HP - 1:HP, :],
                                    in_=ztw[:, :])

            # --- stage 0: sign(x) -> padded fp8 planes in DRAM, 4 img/tile ---
            # xf shares the "sh" buffer (used later by conv0 input / conv2 out)
            if '0' in stages:
                for bb in range(NB):
                    for rb in range(H // 128):
                        xf = shpool.tile([128, B, W], F32, tag="sh")
                        src = x_in[bb * B:(bb + 1) * B,
                                   rb * 128:(rb + 1) * 128, :]
                        nc.sync.dma_start(out=xf[:, :, :],
                                          in_=src.transpose([1, 0, 2]))
                        xp = xpool.tile([128, B, WP], FP8, tag="xp")
                        nc.vector.memset(xp[:, :, :], 0.0)
                        nc.scalar.activation(xp[:, :, 1:W + 1], xf[:, :, :],
                                             AF.Sign)
                        dst = xs_d[bb * B:(bb + 1) * B,
                                   rb * 128 + 1:(rb + 1) * 128 + 1, :]
                        nc.scalar.dma_start(out=dst.transpose([1, 0, 2]),
                                            in_=xp[:, :, :])

            for b in range(NB):
                if 'A' not in stages and 'B' not in stages \
                        and 'C' not in stages:
                    break
                # SBUF-resident padded feature planes for this batch
                # (memset free size is ISA-capped at 64K elements -> halves)
                h0p = h0pool.tile([128, HP, WP], FP8, tag="h0")
                nc.vector.memset(h0p[:, 0:HP // 2, :], 0.0)
                nc.vector.memset(h0p[:, HP // 2:HP, :], 0.0)
                h1p = h1pool.tile([128, HP, WP], FP8, tag="h1")
                nc.vector.memset(h1p[:, 0:HP // 2, :], 0.0)
                nc.vector.memset(h1p[:, HP // 2:HP, :], 0.0)

                # ---- stage A: conv0 (1 -> 32ch); K=4 imgs, DR tap pairs ----
                # whole padded sign(x) plane via ONE contiguous DMA
                if 'A' in stages:
                    xsf = shpool.tile([B, HP, WP], FP8, tag="sh")
                    nc.sync.dma_start(out=xsf[:, :, :],
                                      in_=xs_d[b * B:(b + 1) * B, :, :])
                    with tc.For_i(0, H // RPI) as j:
                        ps = pspool.tile([128, 512 * NBANK], F32,
                                         tag="ps", name="ps")
                        for fi in range(NBANK):
                            _conv_taps_dr(nc, ps[:, fi * 512:(fi + 1) * 512],
                                          s0t, xsf, RPI * j + 2 * fi,
                                          npart=B)
                        nc.scalar.activation(
                            h0p[:, ds(RPI * j + 1, RPI), 1:W + 1],
                            ps[:, :].rearrange("p (a c) -> p a c", a=RPI),
                            AF.Sign)

                # ---- stage B: conv1 (32 -> 32ch); fp8 DoubleRow tap pairs --
                if 'B' in stages:
                    with tc.For_i(0, H // RPI) as j:
                        ps = pspool.tile([128, 512 * NBANK], F32,
                                         tag="ps", name="ps")
                        for fi in range(NBANK):
                            _conv_taps_dr(nc, ps[:, fi * 512:(fi + 1) * 512],
                                          s1t, h0p, RPI * j + 2 * fi)
                        nc.scalar.activation(
                            h1p[:, ds(RPI * j + 1, RPI), 1:W + 1],
                            ps[:, :].rearrange("p (a c) -> p a c", a=RPI),
                            AF.Sign)

                # ---- stage C: conv2 (32 -> 1ch); M zero-padded to 128 ----
                # ot shares the "sh" buffer (conv0 input is consumed by now)
                if 'C' in stages:
                    with tc.For_i(0, H // RC) as jo:
                        ot = shpool.tile([B, RC, W], F16, tag="sh")
                        with tc.For_i(0, RC // RPI) as ji:
                            ps = pspool.tile([128, 512 * NBANK], F32,
                                             tag="ps", name="ps")
                            for fi in range(NBANK):
                                _conv_taps_dr(
                                    nc, ps[:, fi * 512:(fi + 1) * 512],
                                    s2t, h1p, RC * jo + RPI * ji + 2 * fi)
                            nc.scalar.activation(
                                ot[:, ds(RPI * ji, RPI), :],
                                ps[0:B, :].rearrange("p (a c) -> p a c",
                                                     a=RPI),
                                AF.Identity)
                        nc.scalar.dma_start(
                            out=out_d[b, :, ds(RC * jo, RC), :],
                            in_=ot[:, :, :])
    nc.compile()
    return nc


def _host_weights(w0, w1, w2):
    """Pack sign(w) into fp8 stationary matrices. tap index t = dy*3+dx."""
    w0s = np.sign(np.asarray(w0, np.float32))  # [32,1,3,3]
    w1s = np.sign(np.asarray(w1, np.float32))  # [32,32,3,3]
    w2s = np.sign(np.asarray(w2, np.float32))  # [1,32,3,3]
    s0 = np.zeros((B, 9, 128), np.float32)
    s1 = np.zeros((128, 9, 128), np.float32)
    s2 = np.zeros((128, 9, 128), np.float32)
    for g in range(B):
        for t, (dy, dx) in enumerate(TAPS):
            # conv0: out[m=(g,co)] += s0[k=g, t, m] * xs[k, pix+off_t]
            s0[g, t, g * 32:(g + 1) * 32] = w0s[:, 0, dy, dx]
            s1[g * 32:(g + 1) * 32, t, g * 32:(g + 1) * 32] = \
                w1s[:, :, dy, dx].T  # [ci, co]
            s2[g * 32:(g + 1) * 32, t, g] = w2s[0, :, dy, dx]
    return (s0.astype(ml_dtypes.float8_e4m3),
            s1.astype(ml_dtypes.float8_e4m3),
            s2.astype(ml_dtypes.float8_e4m3))


_NC_CACHE = {}


def kernel(x, w0, w1, w2):
    if "nc" not in _NC_CACHE:
        _NC_CACHE["nc"] = _build_program()
    nc = _NC_CACHE["nc"]
    s0, s1, s2 = _host_weights(w0, w1, w2)
    x = np.asarray(x, np.float32).reshape(64, H, W)
    in_maps = [
        {"x": np.ascontiguousarray(x[i * IMG_PER_CORE:(i + 1) * IMG_PER_CORE]),
         "s0": s0, "s1": s1, "s2": s2}
        for i in range(N_CORES)
    ]
    res = run_bass_kernel_spmd(nc, in_maps, list(range(N_CORES)))
    out = np.stack([np.asarray(res.results[i]["out"])
                    for i in range(N_CORES)]).astype(np.float32)
    return out.reshape(64, 1, H, W)
